# revision 12
# baseline (speedup 1.0000x reference)
"""Causal self-attention (B=4, S=2048, D=1024, single head) on 8 TRN2 cores.

Sharding: core c -> batch b = c//2, parity h = c%2. Core handles q-tiles
2s+h (s=0..7) AND computes K/V projections only for its own KEY half
(keys h*1024 .. h*1024+1023). The pair exchanges K^T/V halves with a
rank-ordered pair AllGather through pair-shared HBM, so both cores hold
the full K^T/V with half0 always first — the SPMD program never needs to
know its own parity; parity lives in the data (inputs + causal mask).

All matmuls run in bf16 (1 PE cycle/row, same rate as f32r) with fp32
PSUM accumulation; bf16 halves SBUF footprint, DMA traffic and the
pair-exchange bytes. V is produced directly in [s, e] layout (stationary
x^T tile, moving Wv^T) so the baseline's V-transpose pass disappears;
Q^T stays resident in SBUF (no DRAM spill).

Per-core PE work: 3x65.5k (K/V/Q proj) + 73.7k (scores) + 9.2k (W^T)
+ 73.7k (PV) ~= 353k rows ~= 147us at 2.4GHz, vs ~513k for the baseline.
"""
import os
import sys

import numpy as np

for _p in ("/opt/trn_rl_repo", "/root/.axon_site/_ro/trn_rl_repo"):
    if os.path.isdir(_p) and _p not in sys.path:
        sys.path.insert(0, _p)

import concourse.bass as bass
import concourse.mybir as mybir
import concourse.tile as tile
from concourse.bass_utils import run_bass_kernel_spmd

B, S, D = 4, 2048, 1024
P = 128
SCALE = 1.0 / float(np.sqrt(D))
F32 = mybir.dt.float32
BF16 = mybir.dt.bfloat16
NCORES = 8
PAIRS = [[0, 1], [2, 3], [4, 5], [6, 7]]
BF16NP = mybir.dt.np(mybir.dt.bfloat16)


def _legalize_single_wait(nc):
    """Walrus in this image encodes at most one sync wait per instruction.
    Split each multi-wait instruction into (n-1) prepended same-engine
    NoOps carrying one wait each (identical blocking semantics on an
    in-order engine)."""
    for fn in nc.m.functions:
        for block in fn.blocks:
            out = []
            for inst in block.instructions:
                si = inst.sync_info
                if si is not None and len(si.on_wait) > 1:
                    waits = list(si.on_wait)
                    for w in waits[:-1]:
                        out.append(mybir.InstNoOp(
                            name=nc.get_next_instruction_name(),
                            engine=inst.engine,
                            sync_info=mybir.SyncInfo(on_wait=[w],
                                                     on_update=[]),
                            bass_nofuse=True,
                            text_hint="waitsplit",
                        ))
                    inst.sync_info = mybir.SyncInfo(
                        on_wait=[waits[-1]], on_update=list(si.on_update))
                out.append(inst)
            try:
                block.instructions[:] = out
            except TypeError:
                block.instructions = out


def _build_program(reps=1, legalize=True):
    nc = bass.Bass("TRN2", target_bir_lowering=False, debug=False,
                   num_devices=NCORES)

    xth = nc.dram_tensor("xth", [D, 1024], BF16, kind="ExternalInput").ap()
    xqh = nc.dram_tensor("xqh", [D, 1024], BF16, kind="ExternalInput").ap()
    wqt = nc.dram_tensor("wqt", [D, D], BF16, kind="ExternalInput").ap()
    wkt = nc.dram_tensor("wkt", [D, D], BF16, kind="ExternalInput").ap()
    wvt = nc.dram_tensor("wvt", [D, D], BF16, kind="ExternalInput").ap()
    mask = nc.dram_tensor("mask", [P, 16 * P], BF16, kind="ExternalInput").ap()
    ident = nc.dram_tensor("ident", [P, P], BF16, kind="ExternalInput").ap()
    out = nc.dram_tensor("out", [1024, D], F32, kind="ExternalOutput").ap()

    # pair-exchange staging (own half) and gathered (both halves) buffers
    kstg = nc.dram_tensor("kstg", [P, 8 * 1024], BF16).ap()
    kgth = nc.dram_tensor("kgth", [2 * P, 8 * 1024], BF16).ap()
    # V exchange split in two so the front half (s-tiles 0-3) lands
    # before the first PV needs it
    vstg2 = [nc.dram_tensor(f"vstg{i}", [P, 4 * 1024], BF16).ap()
             for i in range(2)]
    vgth2 = [nc.dram_tensor(f"vgth{i}", [2 * P, 4 * 1024], BF16).ap()
             for i in range(2)]

    with tile.TileContext(nc) as tc:
        from contextlib import ExitStack

        persist = ExitStack()
        kt_pool = persist.enter_context(tc.tile_pool(name="ktp", bufs=1))
        v_pool = persist.enter_context(tc.tile_pool(name="vp", bufs=1))
        q_pool = persist.enter_context(tc.tile_pool(name="qp", bufs=1))
        const_pool = persist.enter_context(tc.tile_pool(name="cst", bufs=1))

        # kt_h[half][p, c*1024+u] = K^T[e=c*128+p, key=half*1024+u]
        kt_h = [kt_pool.tile([P, 8 * 1024], BF16, name=f"kt{h}", tag=f"kt{h}")
                for h in range(2)]
        # vv_h[half][p, t*1024+e] = V[s=half*1024+t*128+p, e]
        vv_h = [v_pool.tile([P, 8 * 1024], BF16, name=f"vv{h}", tag=f"vv{h}")
                for h in range(2)]
        # qts[p, c*1024+q] = Q^T[e=c*128+p, q(own slot-order)]
        qts = q_pool.tile([P, 8 * 1024], BF16, name="qts", tag="qts")
        mk = const_pool.tile([P, 16 * P], BF16)
        idn = const_pool.tile([P, P], BF16)

        nc.sync.dma_start(out=mk[:], in_=mask)
        nc.sync.dma_start(out=idn[:], in_=ident)

        xth_v = xth.rearrange("(g p) s -> p g s", p=P)   # [128, 8, 1024]
        xqh_v = xqh.rearrange("(g p) q -> p g q", p=P)   # [128, 8, 1024]
        w_vs = {"q": wqt.rearrange("(g p) e -> p g e", p=P),
                "k": wkt.rearrange("(g p) e -> p g e", p=P),
                "v": wvt.rearrange("(g p) e -> p g e", p=P)}

        for _rep in range(reps):
          # =============== phase 1: projections + pair exchange ==========
          with ExitStack() as ph1:
            x_pool = ph1.enter_context(tc.tile_pool(name="xh", bufs=1))
            w_pool = ph1.enter_context(tc.tile_pool(name="wsl", bufs=1))
            stg_pool = ph1.enter_context(tc.tile_pool(name="stg", bufs=3))
            ps_pool = ph1.enter_context(
                tc.tile_pool(name="psA", bufs=4, space="PSUM"))

            xh = x_pool.tile([P, 8 * 1024], BF16, tag="xh")
            nc.sync.dma_start(out=xh[:].rearrange("p (g s) -> p g s", g=8),
                              in_=xth_v)
            wsb = {}
            for pj in ("k", "v", "q"):
                wsb[pj] = w_pool.tile([P, 8 * 1024], BF16, tag=f"w{pj}",
                                      name=f"w{pj}")
            nc.sync.dma_start(
                out=wsb["k"][:].rearrange("p (g e) -> p g e", g=8),
                in_=w_vs["k"])
            xq = x_pool.tile([P, 8 * 1024], BF16, tag="xq")
            nc.sync.dma_start(out=xq[:].rearrange("p (g q) -> p g q", g=8),
                              in_=xqh_v)
            for pj in ("v", "q"):
                nc.sync.dma_start(
                    out=wsb[pj][:].rearrange("p (g e) -> p g e", g=8),
                    in_=w_vs[pj])

            # ---- K^T own half: stationary wk e-tile, moving x^T s-chunks
            for c in range(8):
                pk = [ps_pool.tile([P, 512], F32, tag="ps",
                                   name=f"pk{c}{j}") for j in range(2)]
                for g in range(8):
                    for j in range(2):
                        nc.tensor.matmul(
                            pk[j][:],
                            wsb["k"][:, g * 1024 + c * P:g * 1024 + (c + 1) * P],
                            xh[:, g * 1024 + j * 512:g * 1024 + (j + 1) * 512],
                            start=(g == 0), stop=(g == 7))
                kst = stg_pool.tile([P, 1024], BF16, tag="stg",
                                    name=f"kst{c}")
                for j in range(2):
                    nc.scalar.copy(kst[:, j * 512:(j + 1) * 512], pk[j][:])
                nc.sync.dma_start(out=kstg[:, c * 1024:(c + 1) * 1024],
                                  in_=kst[:])
            nc.gpsimd.collective_compute(
                "AllGather", mybir.AluOpType.bypass, PAIRS,
                ins=[kstg[:, :]], outs=[kgth[:, :]])
            for h in range(2):
                nc.sync.dma_start(out=kt_h[h][:],
                                  in_=kgth[h * P:(h + 1) * P, :])

            # ---- V own half, direct [s, e]: stationary x^T s-tile,
            # ---- moving wv e-chunks; exchange fires per 4-tile group
            for t in range(8):
                pv = [ps_pool.tile([P, 512], F32, tag="ps",
                                   name=f"pv{t}{j}") for j in range(2)]
                for g in range(8):
                    for eh in range(2):
                        nc.tensor.matmul(
                            pv[eh][:],
                            xh[:, g * 1024 + t * P:g * 1024 + (t + 1) * P],
                            wsb["v"][:, g * 1024 + eh * 512:
                                     g * 1024 + (eh + 1) * 512],
                            start=(g == 0), stop=(g == 7))
                vst = stg_pool.tile([P, 1024], BF16, tag="stg",
                                    name=f"vst{t}")
                for eh in range(2):
                    nc.scalar.copy(vst[:, eh * 512:(eh + 1) * 512], pv[eh][:])
                grp, tl = t // 4, t % 4
                nc.sync.dma_start(
                    out=vstg2[grp][:, tl * 1024:(tl + 1) * 1024],
                    in_=vst[:])
                if tl == 3:
                    nc.gpsimd.collective_compute(
                        "AllGather", mybir.AluOpType.bypass, PAIRS,
                        ins=[vstg2[grp][:, :]], outs=[vgth2[grp][:, :]])
                    for h in range(2):
                        nc.sync.dma_start(
                            out=vv_h[h][:, grp * 4096:(grp + 1) * 4096],
                            in_=vgth2[grp][h * P:(h + 1) * P, :])

            # ---- Q^T own queries (slot order), straight to SBUF
            for c in range(8):
                pq = [ps_pool.tile([P, 512], F32, tag="ps",
                                   name=f"pq{c}{j}") for j in range(2)]
                for g in range(8):
                    for j in range(2):
                        nc.tensor.matmul(
                            pq[j][:],
                            wsb["q"][:, g * 1024 + c * P:g * 1024 + (c + 1) * P],
                            xq[:, g * 1024 + j * 512:g * 1024 + (j + 1) * 512],
                            start=(g == 0), stop=(g == 7))
                for j in range(2):
                    nc.vector.tensor_copy(
                        qts[:, c * 1024 + j * 512:c * 1024 + (j + 1) * 512],
                        pq[j][:])

          # ================= phase 2: attention =================
          with ExitStack() as ph2:
              we_pool = ph2.enter_context(tc.tile_pool(name="wex", bufs=2))
              wt_sb_pool = ph2.enter_context(tc.tile_pool(name="wtsb", bufs=2))
              o_pool = ph2.enter_context(tc.tile_pool(name="osb", bufs=2))
              st_pool = ph2.enter_context(tc.tile_pool(name="stat", bufs=8))
              psc_pool = ph2.enter_context(
                  tc.tile_pool(name="psS", bufs=3, space="PSUM"))
              pso_pool = ph2.enter_context(
                  tc.tile_pool(name="psO", bufs=2, space="PSUM"))
              pst_pool = ph2.enter_context(
                  tc.tile_pool(name="psW", bufs=1, space="PSUM"))

              for s in range(8):
                  E = 2 * (s + 1)          # k-tiles of 128
                  L = E * P                # k-cols: 256..2048

                  # unsafe softmax: |scores| <~ 6 sigma so exp() is f32-safe
                  # without the running-max pass; the causal mask (-1e30)
                  # folds into PSUM via an identity-stationary matmul, and
                  # exp reads PSUM directly, emitting row-sums (accum_out).
                  wexp = we_pool.tile([P, 2048], BF16, tag="wex")
                  ellp = st_pool.tile([P, 4], F32, tag="ellp")
                  nch = (L + 511) // 512
                  for kch in range(nch):
                      w = min(512, L - kch * 512)
                      h2, loc = kch // 2, 512 * (kch % 2)
                      ps = psc_pool.tile([P, 512], F32, tag="sc",
                                         name=f"sc{s}{kch}")
                      qsl = [qts[:, g * 1024 + s * P:g * 1024 + (s + 1) * P]
                             for g in range(8)]
                      if kch < nch - 1:
                          for g in range(8):
                              nc.tensor.matmul(
                                  ps[:, :w], qsl[g],
                                  kt_h[h2][:, g * 1024 + loc:
                                           g * 1024 + loc + w],
                                  start=(g == 0), stop=(g == 7))
                      else:
                          if w > 256:
                              for g in range(8):
                                  nc.tensor.matmul(
                                      ps[:, :w - 256], qsl[g],
                                      kt_h[h2][:, g * 1024 + loc:
                                               g * 1024 + loc + w - 256],
                                      start=(g == 0), stop=(g == 7))
                          for g in range(8):
                              nc.tensor.matmul(
                                  ps[:, w - 256:w], qsl[g],
                                  kt_h[h2][:, g * 1024 + loc + w - 256:
                                           g * 1024 + loc + w],
                                  start=(g == 0), stop=False)
                          nc.tensor.matmul(
                              ps[:, w - 256:w], idn[:],
                              mk[:, s * 256:(s + 1) * 256],
                              start=False, stop=True)
                      nc.scalar.activation(
                          wexp[:, kch * 512:kch * 512 + w], ps[:, :w],
                          mybir.ActivationFunctionType.Exp,
                          accum_out=ellp[:, kch:kch + 1])

                  ell = st_pool.tile([P, 1], F32, tag="st")
                  nc.vector.reduce_sum(ell[:], ellp[:, :nch],
                                       axis=mybir.AxisListType.X)
                  rinv = st_pool.tile([P, 1], F32, tag="st")
                  nc.vector.reciprocal(rinv[:], ell[:])

                  # transpose W (pack 4 tiles per PSUM bank)
                  wt_sb = wt_sb_pool.tile([P, 2048], BF16, tag="wtsb")
                  for bk in range((E + 3) // 4):
                      ntb = min(4, E - 4 * bk)
                      ptw = pst_pool.tile([P, 512], BF16, tag="ptw")
                      for t4 in range(ntb):
                          ki = 4 * bk + t4
                          nc.tensor.transpose(
                              ptw[:, t4 * P:(t4 + 1) * P],
                              wexp[:, ki * P:(ki + 1) * P], idn[:])
                      nc.vector.tensor_copy(
                          wt_sb[:, 4 * bk * P:4 * bk * P + ntb * P],
                          ptw[:, :ntb * P])

                  # PV
                  po = pso_pool.tile([P, 1024], F32, tag="po")
                  for ki in range(E):
                      h2, t = ki // 8, ki % 8
                      for eh in range(2):
                          nc.tensor.matmul(
                              po[:, eh * 512:(eh + 1) * 512],
                              wt_sb[:, ki * P:(ki + 1) * P],
                              vv_h[h2][:, t * D + eh * 512:
                                       t * D + (eh + 1) * 512],
                              start=(ki == 0), stop=(ki == E - 1))

                  o_sb = o_pool.tile([P, 1024], F32, tag="osb")
                  nc.vector.tensor_scalar_mul(o_sb[:], po[:], rinv[:])
                  nc.sync.dma_start(out=out[s * P:(s + 1) * P, :], in_=o_sb[:])

        persist.close()

    if legalize:
        _legalize_single_wait(nc)
    return nc


_NC = {}


def _get_program(reps=1):
    if reps not in _NC:
        _NC[reps] = _build_program(reps)
    return _NC[reps]


def _make_mask(h):
    i = np.arange(P)[:, None]
    j2 = np.arange(256)[None, :]
    blk = np.where(j2 <= h * P + i, 0.0, -1e30).astype(np.float32)
    return np.tile(blk, (1, 8)).copy()


def _make_in_maps(x, Wq, Wk, Wv):
    x = np.asarray(x, dtype=np.float32)
    xbf = x.astype(BF16NP)
    wqt = np.ascontiguousarray(
        (np.asarray(Wq, dtype=np.float32).T * np.float32(SCALE))
    ).astype(BF16NP)
    wkt = np.ascontiguousarray(
        np.asarray(Wk, dtype=np.float32).T).astype(BF16NP)
    wvt = np.ascontiguousarray(
        np.asarray(Wv, dtype=np.float32).T).astype(BF16NP)
    ident = np.eye(P, dtype=np.float32).astype(BF16NP)
    masks = [_make_mask(0).astype(BF16NP), _make_mask(1).astype(BF16NP)]

    in_maps = []
    for c in range(NCORES):
        b, h = c // 2, c % 2
        xt = xbf[b].T  # [D, S] view
        xth = np.ascontiguousarray(xt[:, h * 1024:(h + 1) * 1024])
        own = np.concatenate([np.arange((2 * s + h) * P, (2 * s + h + 1) * P)
                              for s in range(8)])
        xqh = np.ascontiguousarray(xt[:, own])
        in_maps.append({"xth": xth, "xqh": xqh, "wqt": wqt, "wkt": wkt,
                        "wvt": wvt, "mask": masks[h], "ident": ident})
    return in_maps


def kernel(x, Wq, Wk, Wv, _trace=False):
    in_maps = _make_in_maps(x, Wq, Wk, Wv)
    nc = _get_program()
    res = run_bass_kernel_spmd(nc, in_maps, list(range(NCORES)),
                               trace=_trace)

    out = np.empty((B, S, D), dtype=np.float32)
    for c in range(NCORES):
        b, h = c // 2, c % 2
        o = res.results[c]["out"]
        for s in range(8):
            out[b, (2 * s + h) * P:(2 * s + h + 1) * P, :] = \
                o[s * P:(s + 1) * P, :]
    if _trace:
        return out, res
    return out


if __name__ == "__main__":
    rng = np.random.default_rng(0)
    xs = rng.standard_normal((B, S, D), dtype=np.float32)
    ws = [rng.standard_normal((D, D), dtype=np.float32) * SCALE
          for _ in range(3)]
    o = kernel(xs, *ws)
    print("kernel ran, out shape", o.shape, "finite:", np.isfinite(o).all())
